# revision 38
# baseline (speedup 1.0000x reference)
"""Causal self-attention (B=2, T=2048, C=1024, H=16) on 8 trn2 NeuronCores.

Sharding: tensor-parallel over heads — 2 heads per core. Each core computes
q/k/v projections for its 2 heads (from a replicated transposed x), causal
attention for those heads, and a partial out-projection [B*T, C]; the host
sums the 8 partials and adds the output bias.

Layouts (per core):
  xT    [1024, 4096]  x transposed (c on partitions), host-prepared, bf16
  qT/kT [128, 4096]   head-dim-major (2 heads x 64 dims on partitions)
  v     natural [t, d] per head via PE transpose of vT
  S^T   [k, q] tiles from matmul(lhsT=kT, rhs=qT); softmax runs without the
        max-subtraction (scores are O(few)), the sum over k rides as a
        ones-column in the PV matmul, normalization divides at the end.
"""

import sys

for _p in ("/opt/trn_rl_repo", "/opt/pypackages"):
    if _p not in sys.path:
        sys.path.append(_p)

from contextlib import ExitStack

import numpy as np
import ml_dtypes

import concourse.bass as bass
import concourse.tile as tile
from concourse import bacc, mybir
from concourse.bass import ts, ds
from concourse.bass_utils import run_bass_kernel_spmd

BF16 = ml_dtypes.bfloat16
F32 = mybir.dt.float32
MBF16 = mybir.dt.bfloat16
AF = mybir.ActivationFunctionType

B, T, C, H = 2, 2048, 1024, 16
HD = C // H              # 64
NCORES = 8
HPC = H // NCORES        # 2 heads per core
BT = B * T               # 4096
SCALE = 1.0 / np.sqrt(HD)
NCO = C // 128           # 8 contraction tiles
NPW = BT // 512          # 8 projection windows
NQW = T // 512           # 4 q-windows per batch
NKT = T // 128           # 16 k-tiles per batch
NTT = BT // 128          # 32 t-tiles


def build_nc(dbg=False):
    nc = bacc.Bacc("TRN2", target_bir_lowering=False, debug=False)

    # window-pair-major: each x chunk loads as a contiguous 256KB block with
    # 2KB per partition line (full DMA efficiency; 1KB lines run at ~160GB/s)
    xT_d = nc.dram_tensor(
        "xT", [NPW // 2, NCO, 128, 1024], MBF16, kind="ExternalInput"
    ).ap()
    wq_d = nc.dram_tensor("wq", [128, NCO, 128], MBF16, kind="ExternalInput").ap()
    wk_d = nc.dram_tensor("wk", [128, NCO, 128], MBF16, kind="ExternalInput").ap()
    wv_d = nc.dram_tensor("wv", [128, NCO, 128], MBF16, kind="ExternalInput").ap()
    bq_d = nc.dram_tensor("bq", [128, 1], F32, kind="ExternalInput").ap()
    bk_d = nc.dram_tensor("bk", [128, 1], F32, kind="ExternalInput").ap()
    wo_d = nc.dram_tensor("wo", [128, C], MBF16, kind="ExternalInput").ap()
    mk_d = nc.dram_tensor("mask", [128, 4, 512], MBF16, kind="ExternalInput").ap()
    out_d = nc.dram_tensor("part", [NTT, 128, C], MBF16, kind="ExternalOutput").ap()

    with tile.TileContext(nc) as tc, ExitStack() as ctx:
        per = ctx.enter_context(tc.tile_pool(name="persist", bufs=1))
        xT = [
            per.tile([128, BT], MBF16, tag=f"xT{i}", name=f"xT{i}") for i in range(NCO)
        ]
        wq = per.tile([128, NCO, 128], MBF16, tag="wq")
        wk = per.tile([128, NCO, 128], MBF16, tag="wk")
        wv = per.tile([128, NCO, 128], MBF16, tag="wv")
        bq = per.tile([128, 1], F32, tag="bq")
        bk = per.tile([128, 1], F32, tag="bk")
        wo = per.tile([128, C], MBF16, tag="wo")
        # head-B rows of wo re-based to partitions 0:64: the last window's
        # out-projection runs per-head (K=64) straight from the tb tiles, so
        # no cross-partition hop DMA sits on the final critical path
        woB = per.tile([64, C], MBF16, tag="woB")
        mk = per.tile([128, 4, 512], MBF16, tag="mk")
        # per-window q operand, zero-padded per head so score matmuls run at
        # full K=128 (plane 0: rows 64:128 are zero; plane 1: rows 0:64 are
        # zero) — every matmul in the kernel then uses the same 128x128
        # tiling mode and the PE never pays a mode-switch drain.
        # Three buffers cover the window lifetimes of the processing order.
        qz = [
            per.tile([128, 2, 512], MBF16, tag=f"qz{i}", name=f"qz{i}")
            for i in range(3)
        ]
        kT = per.tile([128, BT], MBF16, tag="kT")
        # v natural, both heads in one tile ([:, j, h, :]), ones-column at
        # col 64 (sumexp rides the PV matmul) — one strided copy evacuates
        # both heads of a t-tile in a single op
        vAB = per.tile([128, NKT * B, 2, 72], MBF16, tag="vAB")
        # normalized attn out, head-major, one tile per 512-window so the
        # out-projection of window g only depends on window g's writers
        aT = [
            per.tile([128, 512], MBF16, tag=f"aT{g}", name=f"aT{g}")
            for g in range(NPW)
        ]

        # --- input DMAs. Two HWDGE rings: sync (qSP) carries ALL of x, issued
        # upfront in window order so the stream never starves the projections
        # (the rings drain at full SDMA parallelism); scalar (qAct) carries the
        # small weights first and is then free for output writes + SBUF hops —
        # keeping outputs off the x ring avoids FIFO queuing behind megabytes
        # of input.
        nc.scalar.dma_start(out=wq[:], in_=wq_d)
        nc.scalar.dma_start(out=wk[:], in_=wk_d)
        nc.scalar.dma_start(out=wv[:], in_=wv_d)
        nc.scalar.dma_start(out=bq[:], in_=bq_d)
        nc.scalar.dma_start(out=bk[:], in_=bk_d)
        nc.scalar.dma_start(out=mk[:], in_=mk_d)
        nc.scalar.dma_start(out=wo[:], in_=wo_d)
        for ci in range(NPW // 2):
            for i in range(NCO):
                nc.sync.dma_start(
                    out=xT[i][:, ci * 1024 : ci * 1024 + 1024],
                    in_=xT_d[ci][i],
                )

        nc.vector.memset(vAB[:, :, :, 64:65], 1.0)
        nc.scalar.dma_start(out=woB[:], in_=wo_d[64:128, :])
        for _q in qz:
            nc.gpsimd.memset(_q[64:128, 0, :], 0.0)
            nc.gpsimd.memset(_q[0:64, 1, :], 0.0)
        # dummy matmuls on zeroed tiles: no input deps, so they run while the
        # first DMAs land — fills the startup PE gap and warms the HAM clock
        wrm_a = per.tile([128, 128], MBF16, tag="wrm_a")
        wrm_b = per.tile([128, 512], MBF16, tag="wrm_b")
        nc.vector.memset(wrm_a[:], 0.0)
        nc.vector.memset(wrm_b[:], 0.0)

        # Pipelined emission: per 512-wide window g, project q/k/v (window g),
        # PE-transpose v, then attention for window g (its scores only need
        # q/k windows <= g), then the out-projection of window g-1. ACT's exp
        # work overlaps projection matmuls; PE stays dense (HAM stays warm).
        with (
            tc.tile_pool(name="pps", bufs=2, space="PSUM") as pps,
            tc.tile_pool(name="sps", bufs=2, space="PSUM") as sps,
            tc.tile_pool(name="pvp", bufs=2, space="PSUM") as pvp,
            tc.tile_pool(name="eap", bufs=8) as eap,
            tc.tile_pool(name="rp", bufs=6) as rp,
            tc.tile_pool(name="bp", bufs=6) as bp,
            tc.tile_pool(name="tbp", bufs=8) as tbp,
            tc.tile_pool(name="stp", bufs=10) as stp,
        ):
            def warm_burst(n):
                # dummy matmuls with no input deps: fill PE stalls (DMA waits,
                # normalize chains) and keep the HAM clock-gate at 2.4GHz
                wrm_ps = pps.tile([128, 512], F32, tag="proj", name="wrm_ps")
                for i in range(n):
                    nc.tensor.matmul(
                        wrm_ps[:], wrm_a[:], wrm_b[:], start=True, stop=True
                    )

            warm_burst(12)

            def proj_mm(w_sb, wi):
                ps = pps.tile([128, 512], F32, tag="proj", name="ps")
                for co in range(NCO):
                    nc.tensor.matmul(
                        ps[:],
                        w_sb[:, co, :],
                        xT[co][:, ts(wi, 512)],
                        start=(co == 0),
                        stop=(co == NCO - 1),
                    )
                return ps

            def proj_q(wi, qzt):
                # the two half-adds each cost a full DVE op (time scales with
                # free size, not partitions) — split them across DVE and ACT
                ps = proj_mm(wq, wi)
                nc.vector.tensor_scalar_add(qzt[0:64, 0, :], ps[0:64, :], bq[0:64, 0:1])
                nc.scalar.activation(
                    qzt[64:128, 1, :], ps[64:128, :], AF.Identity,
                    bias=bq[64:128, 0:1],
                )

            def proj(w_sb, b_sb, dest, wi):
                ps = proj_mm(w_sb, wi)
                nc.vector.tensor_scalar_add(dest[:, ts(wi, 512)], ps[:], b_sb[:, 0:1])

            def outproj_one(g, tt, evac="dve", ring=None):
                # one-bank PSUM tiles in the proj ring: outproj evacuation must
                # not sit in the scores ring, where it would block the next
                # window's score matmuls on a DVE backlog. evac: "dve" (bulk,
                # keeps ACT free for exps), "split" (DVE+ACT halves, for tail
                # stretches where ACT has slack), "act" (full ACT — final
                # window, keeps DVE free so the quarter muls/hops pipeline)
                a_sl = aT[g][:, ts(tt - 4 * g, 128)]
                for h_ in range(2):
                    op = pps.tile([128, 512], F32, tag="proj", name=f"op{h_}")
                    nc.tensor.matmul(
                        op[:], a_sl, wo[:, ts(h_, 512)], start=True, stop=True
                    )
                    st = stp.tile([128, 512], MBF16, tag="st")
                    if evac == "split":
                        nc.vector.tensor_copy(out=st[:, 0:256], in_=op[:, 0:256])
                        nc.scalar.activation(st[:, 256:512], op[:, 256:512], AF.Copy)
                    elif evac == "act":
                        nc.scalar.activation(st[:], op[:], AF.Copy)
                    else:
                        nc.vector.tensor_copy(out=st[:], in_=op[:])
                    eng = ring if ring is not None else nc.sync
                    eng.dma_start(out=out_d[tt][:, ts(h_, 512)], in_=st[:])

            def outproj(g):
                for tt in range(4 * g, 4 * g + 4):
                    outproj_one(g, tt)

            def attention(b, w, qzw, filler=None):
                nk = 4 * (w + 1)
                pva = pvp.tile([128, 512], F32, tag="pv", name="pva")
                pvb = pvp.tile([128, 512], F32, tag="pv", name="pvb")
                def emit_pv(jp, ea, eb, c0s):
                    j0 = 2 * jp
                    for (e, h_, pv) in ((ea, 0, pva), (eb, 1, pvb)):
                        for jj, jloc in ((0, j0), (1, j0 + 1)):
                            c0 = c0s[jj]
                            nc.tensor.matmul(
                                pv[0:65, ds(c0, 512 - c0)],
                                vAB[:, b * NKT + jloc, h_, 0:65],
                                e[:, jj, ds(c0, 512 - c0)],
                                start=(jloc == 0),
                                stop=(jloc == nk - 1),
                            )

                pend = None
                for jp in range(nk // 2):
                    j0, j1 = 2 * jp, 2 * jp + 1
                    diag = j0 >= nk - 4
                    # diagonal block i: columns < 128*i are fully masked, so
                    # scores/exp/PV only touch columns [128*i, 512)
                    c0s = [
                        max(0, (jloc - (nk - 4)) * 128) if diag else 0
                        for jloc in (j0, j1)
                    ]
                    # scores: all of head A before head B, so exp(sa) can
                    # start two matmuls earlier — the sa ring slot gates the
                    # next jp's scores through that exp
                    sa = sps.tile([128, 2, 512], F32, tag="s", name="sa")
                    sb_ = sps.tile([128, 2, 512], F32, tag="s", name="sb")
                    for s_ps, h_ in ((sa, 0), (sb_, 1)):
                        for jj, jloc in ((0, j0), (1, j1)):
                            kd = ds(b * T + jloc * 128, 128)
                            c0 = c0s[jj]
                            cw = ds(c0, 512 - c0)
                            nc.tensor.matmul(
                                s_ps[:, jj, cw], kT[:, kd], qzw[:, h_, cw],
                                start=True, stop=True,
                            )
                    es = []
                    for s_ps in (sa, sb_):
                        e = eap.tile([128, 2, 512], MBF16, tag="e")
                        if not diag:
                            nc.scalar.activation(e[:], s_ps[:], AF.Exp, scale=float(SCALE))
                        else:
                            for jj, jloc in ((0, j0), (1, j1)):
                                i0 = jloc - (nk - 4)
                                cw = ds(c0s[jj], 512 - c0s[jj])
                                nc.scalar.activation(
                                    e[:, jj, cw], s_ps[:, jj, cw], AF.Exp,
                                    scale=float(SCALE),
                                )
                                nc.vector.tensor_mul(
                                    e[:, jj, cw], e[:, jj, cw], mk[:, i0, cw]
                                )
                        es.append(e)
                    # software pipeline: PV of stage jp-1 issues after scores of
                    # stage jp, so exp latency never blocks the PE stream
                    if pend is not None:
                        emit_pv(*pend)
                    pend = (jp, es[0], es[1], c0s)
                # fill the final exp's latency with independent PE work
                if filler is not None:
                    filler()
                emit_pv(*pend)
                return pva, pvb

            def normalize(b, w, pva, pvb, cols=None):
                # rows 0..63 head dims, row 64 sumexp
                g = NQW * b + w
                c0, cn = cols if cols else (0, 512)
                cs = ds(c0, cn)
                for (pv, hlo) in ((pva, 0), (pvb, 64)):
                    # custom-DVE recip misreads PSUM on HW: copy to SBUF first
                    # (on DVE — an ACT copy would delay the next window's exps)
                    sm = rp.tile([1, 512], F32, tag="sm", name="sm")
                    nc.vector.tensor_copy(out=sm[0:1, 0:cn], in_=pv[64:65, cs])
                    rc = rp.tile([1, 512], F32, tag="rc", name="rc")
                    nc.vector.reciprocal_approx_fast(out=rc[0:1, 0:cn], in_=sm[0:1, 0:cn])
                    bc = bp.tile([64, 512], F32, tag="bc", name="bc")
                    nc.gpsimd.partition_broadcast(bc[:, 0:cn], rc[0:1, 0:cn], channels=64)
                    if hlo == 0:
                        nc.vector.tensor_mul(aT[g][0:64, cs], pv[0:64, cs], bc[:, 0:cn])
                    else:
                        tb = tbp.tile([64, 512], MBF16, tag="tb")
                        nc.vector.tensor_mul(tb[:, 0:cn], pv[0:64, cs], bc[:, 0:cn])
                        # head B lives on partitions 64..127 of aT; DVE can't
                        # cross partitions, so hop through an SBUF->SBUF DMA.
                        nc.sync.dma_start(out=aT[g][64:128, cs], in_=tb[:, 0:cn])

            # qz buffer per window, chosen so lifetimes (proj -> attention,
            # in processing order 0,1,2,3,5,6,7,4) never overlap per buffer
            QZB = {0: 0, 1: 1, 2: 0, 3: 1, 5: 0, 6: 1, 7: 0, 4: 2}

            def do_proj(g):
                proj_q(g, qz[QZB[g]])
                proj(wk, bk, kT, g)
                # v in natural [t, d] layout directly: per t-tile, accumulate
                # xT-slice.T @ wv over the 8 c-slices (N=128 MMs, FWL-friendly).
                # No PE transpose -> no tiling-mode switches; bv is folded into
                # the output on the host (softmax weights sum to 1).
                for j in range(4 * g, 4 * g + 4):
                    vp = pps.tile([128, 128], F32, tag="proj", name="vp")
                    for co in range(NCO):
                        nc.tensor.matmul(
                            vp[:], xT[co][:, ts(j, 128)], wv[:, co, :],
                            start=(co == 0), stop=(co == NCO - 1),
                        )
                    nc.vector.tensor_copy(
                        out=vAB[:, j, :, 0:64],
                        in_=vp.rearrange("p (h d) -> p h d", h=2),
                    )

            # Window processing order: batch-1's w0 is projected in its slot
            # (later b1 windows attend to its k/v) but its attention runs LAST:
            # it's the cheapest window (4 k-tiles), so the big b1w3
            # normalize/out-projection hides inside it and the kernel tail
            # after the final matmul stays short.
            glast = NQW  # b1w0
            order = [g for g in range(NPW) if g != glast] + [glast]
            do_proj(order[0])
            prev = None
            for idx, g in enumerate(order[:-1]):
                # out-projection of the previous window fills the final exp
                # latency inside attention; its PSUM->SBUF copies also reach
                # the DVE queue ahead of this window's normalize chain
                filler = (lambda gp=prev: outproj(gp)) if prev is not None else None
                pva, pvb = attention(g // NQW, g % NQW, qz[QZB[g]], filler=filler)
                # project the next window(s) BEFORE normalize(g): the DVE
                # queue then serves the q/k bias-adds and v copies the next
                # window's matmuls wait on ahead of g's normalize chain
                nxt = order[idx + 1]
                if nxt != glast:
                    do_proj(nxt)
                if nxt == glast + 1:
                    do_proj(glast)
                normalize(g // NQW, g % NQW, pva, pvb)
                prev = g
            # last processed window (b1w0): interleave its normalize and
            # out-projection in 128-col quarters so the tail after the final
            # matmul is just one evacuation + DMA
            pva, pvb = attention(glast // NQW, glast % NQW, qz[QZB[glast]],
                                 filler=None)
            # staged final normalize: both heads' recip-broadcast chains start
            # immediately after the last PV; the previous window's
            # out-projection (gated only by its own normalize, already done)
            # keeps the PE busy under them
            bcs = []
            for (pv, hlo) in ((pva, 0), (pvb, 64)):
                sm = rp.tile([1, 512], F32, tag="sm", name="sml")
                nc.vector.tensor_copy(out=sm[0:1, :], in_=pv[64:65, :])
                rc = rp.tile([1, 512], F32, tag="rc", name="rcl")
                nc.vector.reciprocal_approx_fast(out=rc[0:1, :], in_=sm[0:1, :])
                bc = bp.tile([64, 512], F32, tag="bc", name="bcl")
                nc.gpsimd.partition_broadcast(bc[:, :], rc[0:1, :], channels=64)
                bcs.append(bc)
            for tt in range(4 * prev, 4 * prev + 4):
                outproj_one(prev, tt, evac="split")
            for qtr in range(4):
                cs = ds(128 * qtr, 128)
                nc.vector.tensor_mul(aT[glast][0:64, cs], pva[0:64, cs], bcs[0][:, cs])
                tb = tbp.tile([64, 512], MBF16, tag="tb")
                nc.vector.tensor_mul(tb[:, 0:128], pvb[0:64, cs], bcs[1][:, cs])
                nc.sync.dma_start(out=aT[glast][64:128, cs], in_=tb[:, 0:128])
                outproj_one(glast, 4 * glast + qtr, evac="act",
                            ring=(nc.scalar if qtr % 2 else nc.sync))

        if dbg:
            for name, t in (("kTd", kT),):
                d = nc.dram_tensor(name, [128, BT], MBF16, kind="ExternalOutput").ap()
                nc.sync.dma_start(out=d, in_=t[:])
            aTd = nc.dram_tensor("aTd", [128, BT], MBF16, kind="ExternalOutput").ap()
            for g in range(NPW):
                nc.sync.dma_start(out=aTd[:, ts(g, 512)], in_=aT[g][:])
            vABd = nc.dram_tensor(
                "vABd", [128, NKT * B, 2, 65], MBF16, kind="ExternalOutput"
            ).ap()
            nc.sync.dma_start(out=vABd, in_=vAB[:, :, :, 0:65])

    nc.compile()
    return nc


_NC = None


def _get_nc():
    global _NC
    if _NC is None:
        _NC = build_nc()
    return _NC


def _make_in_maps(x, w_qkv, b_qkv, w_out):
    xT = np.ascontiguousarray(
        x.reshape(BT, C).T.reshape(NCO, 128, NPW // 2, 1024).transpose(2, 0, 1, 3)
    ).astype(BF16)
    p = np.arange(128)[:, None]
    f = np.arange(512)[None, :]
    mask = np.stack([(128 * i + p <= f) for i in range(4)], axis=1).astype(BF16)
    in_maps = []
    for i in range(NCORES):
        sl = slice(128 * i, 128 * i + 128)
        m = {
            "xT": xT,
            "wq": np.ascontiguousarray(
                w_qkv[:, sl].reshape(NCO, 128, 128).transpose(1, 0, 2)
            ).astype(BF16),
            "wk": np.ascontiguousarray(
                w_qkv[:, C + 128 * i : C + 128 * i + 128]
                .reshape(NCO, 128, 128)
                .transpose(1, 0, 2)
            ).astype(BF16),
            "wv": np.ascontiguousarray(
                w_qkv[:, 2 * C + 128 * i : 2 * C + 128 * i + 128]
                .reshape(NCO, 128, 128)
                .transpose(1, 0, 2)
            ).astype(BF16),
            "bq": b_qkv[sl].astype(np.float32).reshape(128, 1),
            "bk": b_qkv[C + 128 * i : C + 128 * i + 128].astype(np.float32).reshape(128, 1),
            "wo": np.ascontiguousarray(w_out[sl, :]).astype(BF16),
            "mask": mask,
        }
        in_maps.append(m)
    return in_maps


def run(inputs, trace=False):
    """Returns (y, exec_time_ns_or_None)."""
    x = np.asarray(inputs["x"], dtype=np.float32)
    w_qkv = np.asarray(inputs["w_qkv"], dtype=np.float32)
    b_qkv = np.asarray(inputs["b_qkv"], dtype=np.float32)
    w_out = np.asarray(inputs["w_out"], dtype=np.float32)
    b_out = np.asarray(inputs["b_out"], dtype=np.float32)

    nc = _get_nc()
    in_maps = _make_in_maps(x, w_qkv, b_qkv, w_out)
    res = run_bass_kernel_spmd(nc, in_maps, list(range(NCORES)), trace=trace)
    part = np.zeros((NTT, 128, C), dtype=np.float32)
    for r in res.results:
        part += r["part"]
    # bv is not applied on-device: softmax weights sum to 1, so it adds
    # bv @ w_out to every row — fold it in here with the output bias
    y = part.reshape(BT, C) + (b_out + b_qkv[2 * C :] @ w_out)[None, :]
    return y.reshape(B, T, C).astype(np.float32), res.exec_time_ns


def kernel(**inputs):
    return run(inputs, trace=False)[0]



# revision 39
# speedup vs baseline: 1.0013x; 1.0013x over previous
"""Causal self-attention (B=2, T=2048, C=1024, H=16) on 8 trn2 NeuronCores.

Sharding: tensor-parallel over heads — 2 heads per core. Each core computes
q/k/v projections for its 2 heads (from a replicated transposed x), causal
attention for those heads, and a partial out-projection [B*T, C]; the host
sums the 8 partials and adds the output bias.

Layouts (per core):
  xT    [1024, 4096]  x transposed (c on partitions), host-prepared, bf16
  qT/kT [128, 4096]   head-dim-major (2 heads x 64 dims on partitions)
  v     natural [t, d] per head via PE transpose of vT
  S^T   [k, q] tiles from matmul(lhsT=kT, rhs=qT); softmax runs without the
        max-subtraction (scores are O(few)), the sum over k rides as a
        ones-column in the PV matmul, normalization divides at the end.
"""

import sys

for _p in ("/opt/trn_rl_repo", "/opt/pypackages"):
    if _p not in sys.path:
        sys.path.append(_p)

from contextlib import ExitStack

import numpy as np
import ml_dtypes

import concourse.bass as bass
import concourse.tile as tile
from concourse import bacc, mybir
from concourse.bass import ts, ds
from concourse.bass_utils import run_bass_kernel_spmd

BF16 = ml_dtypes.bfloat16
F32 = mybir.dt.float32
MBF16 = mybir.dt.bfloat16
AF = mybir.ActivationFunctionType

B, T, C, H = 2, 2048, 1024, 16
HD = C // H              # 64
NCORES = 8
HPC = H // NCORES        # 2 heads per core
BT = B * T               # 4096
SCALE = 1.0 / np.sqrt(HD)
NCO = C // 128           # 8 contraction tiles
NPW = BT // 512          # 8 projection windows
NQW = T // 512           # 4 q-windows per batch
NKT = T // 128           # 16 k-tiles per batch
NTT = BT // 128          # 32 t-tiles


def build_nc(dbg=False):
    nc = bacc.Bacc("TRN2", target_bir_lowering=False, debug=False)

    # window-pair-major: each x chunk loads as a contiguous 256KB block with
    # 2KB per partition line (full DMA efficiency; 1KB lines run at ~160GB/s)
    xT_d = nc.dram_tensor(
        "xT", [NPW // 2, NCO, 128, 1024], MBF16, kind="ExternalInput"
    ).ap()
    wq_d = nc.dram_tensor("wq", [128, NCO, 128], MBF16, kind="ExternalInput").ap()
    wk_d = nc.dram_tensor("wk", [128, NCO, 128], MBF16, kind="ExternalInput").ap()
    wv_d = nc.dram_tensor("wv", [128, NCO, 128], MBF16, kind="ExternalInput").ap()
    bq_d = nc.dram_tensor("bq", [128, 1], F32, kind="ExternalInput").ap()
    bk_d = nc.dram_tensor("bk", [128, 1], F32, kind="ExternalInput").ap()
    wo_d = nc.dram_tensor("wo", [128, C], MBF16, kind="ExternalInput").ap()
    mk_d = nc.dram_tensor("mask", [128, 4, 512], MBF16, kind="ExternalInput").ap()
    out_d = nc.dram_tensor("part", [NTT, 128, C], MBF16, kind="ExternalOutput").ap()

    with tile.TileContext(nc) as tc, ExitStack() as ctx:
        per = ctx.enter_context(tc.tile_pool(name="persist", bufs=1))
        xT = [
            per.tile([128, BT], MBF16, tag=f"xT{i}", name=f"xT{i}") for i in range(NCO)
        ]
        wq = per.tile([128, NCO, 128], MBF16, tag="wq")
        wk = per.tile([128, NCO, 128], MBF16, tag="wk")
        wv = per.tile([128, NCO, 128], MBF16, tag="wv")
        bq = per.tile([128, 1], F32, tag="bq")
        bk = per.tile([128, 1], F32, tag="bk")
        wo = per.tile([128, C], MBF16, tag="wo")
        # head-B rows of wo re-based to partitions 0:64: the last window's
        # out-projection runs per-head (K=64) straight from the tb tiles, so
        # no cross-partition hop DMA sits on the final critical path
        woB = per.tile([64, C], MBF16, tag="woB")
        mk = per.tile([128, 4, 512], MBF16, tag="mk")
        # per-window q operand, zero-padded per head so score matmuls run at
        # full K=128 (plane 0: rows 64:128 are zero; plane 1: rows 0:64 are
        # zero) — every matmul in the kernel then uses the same 128x128
        # tiling mode and the PE never pays a mode-switch drain.
        # Three buffers cover the window lifetimes of the processing order.
        qz = [
            per.tile([128, 2, 512], MBF16, tag=f"qz{i}", name=f"qz{i}")
            for i in range(3)
        ]
        kT = per.tile([128, BT], MBF16, tag="kT")
        # v natural, both heads in one tile ([:, j, h, :]), ones-column at
        # col 64 (sumexp rides the PV matmul) — one strided copy evacuates
        # both heads of a t-tile in a single op
        vAB = per.tile([128, NKT * B, 2, 72], MBF16, tag="vAB")
        # normalized attn out, head-major, one tile per 512-window so the
        # out-projection of window g only depends on window g's writers
        aT = [
            per.tile([128, 512], MBF16, tag=f"aT{g}", name=f"aT{g}")
            for g in range(NPW)
        ]

        # --- input DMAs. Two HWDGE rings: sync (qSP) carries ALL of x, issued
        # upfront in window order so the stream never starves the projections
        # (the rings drain at full SDMA parallelism); scalar (qAct) carries the
        # small weights first and is then free for output writes + SBUF hops —
        # keeping outputs off the x ring avoids FIFO queuing behind megabytes
        # of input.
        nc.scalar.dma_start(out=wq[:], in_=wq_d)
        nc.scalar.dma_start(out=wk[:], in_=wk_d)
        nc.scalar.dma_start(out=wv[:], in_=wv_d)
        nc.scalar.dma_start(out=bq[:], in_=bq_d)
        nc.scalar.dma_start(out=bk[:], in_=bk_d)
        nc.scalar.dma_start(out=mk[:], in_=mk_d)
        nc.scalar.dma_start(out=wo[:], in_=wo_d)
        for ci in range(NPW // 2):
            for i in range(NCO):
                nc.sync.dma_start(
                    out=xT[i][:, ci * 1024 : ci * 1024 + 1024],
                    in_=xT_d[ci][i],
                )

        nc.vector.memset(vAB[:, :, :, 64:65], 1.0)
        nc.scalar.dma_start(out=woB[:], in_=wo_d[64:128, :])
        for _q in qz:
            nc.gpsimd.memset(_q[64:128, 0, :], 0.0)
            nc.gpsimd.memset(_q[0:64, 1, :], 0.0)
        # dummy matmuls on zeroed tiles: no input deps, so they run while the
        # first DMAs land — fills the startup PE gap and warms the HAM clock
        wrm_a = per.tile([128, 128], MBF16, tag="wrm_a")
        wrm_b = per.tile([128, 512], MBF16, tag="wrm_b")
        nc.vector.memset(wrm_a[:], 0.0)
        nc.vector.memset(wrm_b[:], 0.0)

        # Pipelined emission: per 512-wide window g, project q/k/v (window g),
        # PE-transpose v, then attention for window g (its scores only need
        # q/k windows <= g), then the out-projection of window g-1. ACT's exp
        # work overlaps projection matmuls; PE stays dense (HAM stays warm).
        with (
            tc.tile_pool(name="pps", bufs=2, space="PSUM") as pps,
            tc.tile_pool(name="sps", bufs=2, space="PSUM") as sps,
            tc.tile_pool(name="pvp", bufs=2, space="PSUM") as pvp,
            tc.tile_pool(name="eap", bufs=8) as eap,
            tc.tile_pool(name="rp", bufs=6) as rp,
            tc.tile_pool(name="bp", bufs=6) as bp,
            tc.tile_pool(name="tbp", bufs=8) as tbp,
            tc.tile_pool(name="stp", bufs=10) as stp,
        ):
            def warm_burst(n):
                # dummy matmuls with no input deps: fill PE stalls (DMA waits,
                # normalize chains) and keep the HAM clock-gate at 2.4GHz
                wrm_ps = pps.tile([128, 512], F32, tag="proj", name="wrm_ps")
                for i in range(n):
                    nc.tensor.matmul(
                        wrm_ps[:], wrm_a[:], wrm_b[:], start=True, stop=True
                    )

            warm_burst(12)

            def proj_mm(w_sb, wi):
                ps = pps.tile([128, 512], F32, tag="proj", name="ps")
                for co in range(NCO):
                    nc.tensor.matmul(
                        ps[:],
                        w_sb[:, co, :],
                        xT[co][:, ts(wi, 512)],
                        start=(co == 0),
                        stop=(co == NCO - 1),
                    )
                return ps

            def proj_q(wi, qzt):
                # the two half-adds each cost a full DVE op (time scales with
                # free size, not partitions) — split them across DVE and ACT
                ps = proj_mm(wq, wi)
                nc.vector.tensor_scalar_add(qzt[0:64, 0, :], ps[0:64, :], bq[0:64, 0:1])
                nc.scalar.activation(
                    qzt[64:128, 1, :], ps[64:128, :], AF.Identity,
                    bias=bq[64:128, 0:1],
                )

            def proj(w_sb, b_sb, dest, wi):
                ps = proj_mm(w_sb, wi)
                nc.vector.tensor_scalar_add(dest[:, ts(wi, 512)], ps[:], b_sb[:, 0:1])

            def outproj_one(g, tt, evac="dve", ring=None):
                # one-bank PSUM tiles in the proj ring: outproj evacuation must
                # not sit in the scores ring, where it would block the next
                # window's score matmuls on a DVE backlog. evac: "dve" (bulk,
                # keeps ACT free for exps), "split" (DVE+ACT halves, for tail
                # stretches where ACT has slack), "act" (full ACT — final
                # window, keeps DVE free so the quarter muls/hops pipeline)
                a_sl = aT[g][:, ts(tt - 4 * g, 128)]
                for h_ in range(2):
                    op = pps.tile([128, 512], F32, tag="proj", name=f"op{h_}")
                    nc.tensor.matmul(
                        op[:], a_sl, wo[:, ts(h_, 512)], start=True, stop=True
                    )
                    st = stp.tile([128, 512], MBF16, tag="st")
                    if evac == "split":
                        nc.vector.tensor_copy(out=st[:, 0:256], in_=op[:, 0:256])
                        nc.scalar.activation(st[:, 256:512], op[:, 256:512], AF.Copy)
                    elif evac == "act":
                        nc.scalar.activation(st[:], op[:], AF.Copy)
                    else:
                        nc.vector.tensor_copy(out=st[:], in_=op[:])
                    eng = ring if ring is not None else nc.sync
                    eng.dma_start(out=out_d[tt][:, ts(h_, 512)], in_=st[:])

            def outproj(g):
                for tt in range(4 * g, 4 * g + 4):
                    outproj_one(g, tt)

            def attention(b, w, qzw, filler=None):
                nk = 4 * (w + 1)
                pva = pvp.tile([128, 512], F32, tag="pv", name="pva")
                pvb = pvp.tile([128, 512], F32, tag="pv", name="pvb")
                def emit_pv(jp, ea, eb, c0s):
                    j0 = 2 * jp
                    for (e, h_, pv) in ((ea, 0, pva), (eb, 1, pvb)):
                        for jj, jloc in ((0, j0), (1, j0 + 1)):
                            c0 = c0s[jj]
                            nc.tensor.matmul(
                                pv[0:65, ds(c0, 512 - c0)],
                                vAB[:, b * NKT + jloc, h_, 0:65],
                                e[:, jj, ds(c0, 512 - c0)],
                                start=(jloc == 0),
                                stop=(jloc == nk - 1),
                            )

                pend = None
                for jp in range(nk // 2):
                    j0, j1 = 2 * jp, 2 * jp + 1
                    diag = j0 >= nk - 4
                    # diagonal block i: columns < 128*i are fully masked, so
                    # scores/exp/PV only touch columns [128*i, 512)
                    c0s = [
                        max(0, (jloc - (nk - 4)) * 128) if diag else 0
                        for jloc in (j0, j1)
                    ]
                    # scores: all of head A before head B, so exp(sa) can
                    # start two matmuls earlier — the sa ring slot gates the
                    # next jp's scores through that exp
                    sa = sps.tile([128, 2, 512], F32, tag="s", name="sa")
                    sb_ = sps.tile([128, 2, 512], F32, tag="s", name="sb")
                    for s_ps, h_ in ((sa, 0), (sb_, 1)):
                        for jj, jloc in ((0, j0), (1, j1)):
                            kd = ds(b * T + jloc * 128, 128)
                            c0 = c0s[jj]
                            cw = ds(c0, 512 - c0)
                            nc.tensor.matmul(
                                s_ps[:, jj, cw], kT[:, kd], qzw[:, h_, cw],
                                start=True, stop=True,
                            )
                    es = []
                    for s_ps in (sa, sb_):
                        e = eap.tile([128, 2, 512], MBF16, tag="e")
                        if not diag:
                            nc.scalar.activation(e[:], s_ps[:], AF.Exp, scale=float(SCALE))
                        else:
                            for jj, jloc in ((0, j0), (1, j1)):
                                i0 = jloc - (nk - 4)
                                cw = ds(c0s[jj], 512 - c0s[jj])
                                nc.scalar.activation(
                                    e[:, jj, cw], s_ps[:, jj, cw], AF.Exp,
                                    scale=float(SCALE),
                                )
                                nc.vector.tensor_mul(
                                    e[:, jj, cw], e[:, jj, cw], mk[:, i0, cw]
                                )
                        es.append(e)
                    # software pipeline: PV of stage jp-1 issues after scores of
                    # stage jp, so exp latency never blocks the PE stream
                    if pend is not None:
                        emit_pv(*pend)
                    pend = (jp, es[0], es[1], c0s)
                # fill the final exp's latency with independent PE work
                if filler is not None:
                    filler()
                emit_pv(*pend)
                return pva, pvb

            def normalize(b, w, pva, pvb, cols=None):
                # rows 0..63 head dims, row 64 sumexp
                g = NQW * b + w
                c0, cn = cols if cols else (0, 512)
                cs = ds(c0, cn)
                for (pv, hlo) in ((pva, 0), (pvb, 64)):
                    # custom-DVE recip misreads PSUM on HW: copy to SBUF first
                    # (on DVE — an ACT copy would delay the next window's exps)
                    sm = rp.tile([1, 512], F32, tag="sm", name="sm")
                    nc.vector.tensor_copy(out=sm[0:1, 0:cn], in_=pv[64:65, cs])
                    rc = rp.tile([1, 512], F32, tag="rc", name="rc")
                    nc.vector.reciprocal_approx_fast(out=rc[0:1, 0:cn], in_=sm[0:1, 0:cn])
                    bc = bp.tile([64, 512], F32, tag="bc", name="bc")
                    nc.gpsimd.partition_broadcast(bc[:, 0:cn], rc[0:1, 0:cn], channels=64)
                    if hlo == 0:
                        nc.vector.tensor_mul(aT[g][0:64, cs], pv[0:64, cs], bc[:, 0:cn])
                    else:
                        tb = tbp.tile([64, 512], MBF16, tag="tb")
                        nc.vector.tensor_mul(tb[:, 0:cn], pv[0:64, cs], bc[:, 0:cn])
                        # head B lives on partitions 64..127 of aT; DVE can't
                        # cross partitions, so hop through an SBUF->SBUF DMA.
                        nc.sync.dma_start(out=aT[g][64:128, cs], in_=tb[:, 0:cn])

            # qz buffer per window, chosen so lifetimes (proj -> attention,
            # in processing order 0,1,2,3,5,6,7,4) never overlap per buffer
            QZB = {0: 0, 1: 1, 2: 0, 3: 1, 5: 0, 6: 1, 7: 0, 4: 2}

            def do_proj(g):
                proj_q(g, qz[QZB[g]])
                proj(wk, bk, kT, g)
                # v in natural [t, d] layout directly: per t-tile, accumulate
                # xT-slice.T @ wv over the 8 c-slices (N=128 MMs, FWL-friendly).
                # No PE transpose -> no tiling-mode switches; bv is folded into
                # the output on the host (softmax weights sum to 1).
                for j in range(4 * g, 4 * g + 4):
                    vp = pps.tile([128, 128], F32, tag="proj", name="vp")
                    for co in range(NCO):
                        nc.tensor.matmul(
                            vp[:], xT[co][:, ts(j, 128)], wv[:, co, :],
                            start=(co == 0), stop=(co == NCO - 1),
                        )
                    nc.scalar.activation(
                        vAB[:, j, :, 0:64],
                        vp.rearrange("p (h d) -> p h d", h=2),
                        AF.Copy,
                    )

            # Window processing order: batch-1's w0 is projected in its slot
            # (later b1 windows attend to its k/v) but its attention runs LAST:
            # it's the cheapest window (4 k-tiles), so the big b1w3
            # normalize/out-projection hides inside it and the kernel tail
            # after the final matmul stays short.
            glast = NQW  # b1w0
            order = [g for g in range(NPW) if g != glast] + [glast]
            do_proj(order[0])
            prev = None
            for idx, g in enumerate(order[:-1]):
                # out-projection of the previous window fills the final exp
                # latency inside attention; its PSUM->SBUF copies also reach
                # the DVE queue ahead of this window's normalize chain
                filler = (lambda gp=prev: outproj(gp)) if prev is not None else None
                pva, pvb = attention(g // NQW, g % NQW, qz[QZB[g]], filler=filler)
                # project the next window(s) BEFORE normalize(g): the DVE
                # queue then serves the q/k bias-adds and v copies the next
                # window's matmuls wait on ahead of g's normalize chain
                nxt = order[idx + 1]
                if nxt != glast:
                    do_proj(nxt)
                if nxt == glast + 1:
                    do_proj(glast)
                normalize(g // NQW, g % NQW, pva, pvb)
                prev = g
            # last processed window (b1w0): interleave its normalize and
            # out-projection in 128-col quarters so the tail after the final
            # matmul is just one evacuation + DMA
            pva, pvb = attention(glast // NQW, glast % NQW, qz[QZB[glast]],
                                 filler=None)
            # staged final normalize: both heads' recip-broadcast chains start
            # immediately after the last PV; the previous window's
            # out-projection (gated only by its own normalize, already done)
            # keeps the PE busy under them
            bcs = []
            for (pv, hlo) in ((pva, 0), (pvb, 64)):
                sm = rp.tile([1, 512], F32, tag="sm", name="sml")
                nc.vector.tensor_copy(out=sm[0:1, :], in_=pv[64:65, :])
                rc = rp.tile([1, 512], F32, tag="rc", name="rcl")
                nc.vector.reciprocal_approx_fast(out=rc[0:1, :], in_=sm[0:1, :])
                bc = bp.tile([64, 512], F32, tag="bc", name="bcl")
                nc.gpsimd.partition_broadcast(bc[:, :], rc[0:1, :], channels=64)
                bcs.append(bc)
            for tt in range(4 * prev, 4 * prev + 4):
                outproj_one(prev, tt, evac="split")
            for qtr in range(4):
                cs = ds(128 * qtr, 128)
                nc.vector.tensor_mul(aT[glast][0:64, cs], pva[0:64, cs], bcs[0][:, cs])
                tb = tbp.tile([64, 512], MBF16, tag="tb")
                nc.vector.tensor_mul(tb[:, 0:128], pvb[0:64, cs], bcs[1][:, cs])
                nc.sync.dma_start(out=aT[glast][64:128, cs], in_=tb[:, 0:128])
                outproj_one(glast, 4 * glast + qtr, evac="act",
                            ring=(nc.scalar if qtr % 2 else nc.sync))

        if dbg:
            for name, t in (("kTd", kT),):
                d = nc.dram_tensor(name, [128, BT], MBF16, kind="ExternalOutput").ap()
                nc.sync.dma_start(out=d, in_=t[:])
            aTd = nc.dram_tensor("aTd", [128, BT], MBF16, kind="ExternalOutput").ap()
            for g in range(NPW):
                nc.sync.dma_start(out=aTd[:, ts(g, 512)], in_=aT[g][:])
            vABd = nc.dram_tensor(
                "vABd", [128, NKT * B, 2, 65], MBF16, kind="ExternalOutput"
            ).ap()
            nc.sync.dma_start(out=vABd, in_=vAB[:, :, :, 0:65])

    nc.compile()
    return nc


_NC = None


def _get_nc():
    global _NC
    if _NC is None:
        _NC = build_nc()
    return _NC


def _make_in_maps(x, w_qkv, b_qkv, w_out):
    xT = np.ascontiguousarray(
        x.reshape(BT, C).T.reshape(NCO, 128, NPW // 2, 1024).transpose(2, 0, 1, 3)
    ).astype(BF16)
    p = np.arange(128)[:, None]
    f = np.arange(512)[None, :]
    mask = np.stack([(128 * i + p <= f) for i in range(4)], axis=1).astype(BF16)
    in_maps = []
    for i in range(NCORES):
        sl = slice(128 * i, 128 * i + 128)
        m = {
            "xT": xT,
            "wq": np.ascontiguousarray(
                w_qkv[:, sl].reshape(NCO, 128, 128).transpose(1, 0, 2)
            ).astype(BF16),
            "wk": np.ascontiguousarray(
                w_qkv[:, C + 128 * i : C + 128 * i + 128]
                .reshape(NCO, 128, 128)
                .transpose(1, 0, 2)
            ).astype(BF16),
            "wv": np.ascontiguousarray(
                w_qkv[:, 2 * C + 128 * i : 2 * C + 128 * i + 128]
                .reshape(NCO, 128, 128)
                .transpose(1, 0, 2)
            ).astype(BF16),
            "bq": b_qkv[sl].astype(np.float32).reshape(128, 1),
            "bk": b_qkv[C + 128 * i : C + 128 * i + 128].astype(np.float32).reshape(128, 1),
            "wo": np.ascontiguousarray(w_out[sl, :]).astype(BF16),
            "mask": mask,
        }
        in_maps.append(m)
    return in_maps


def run(inputs, trace=False):
    """Returns (y, exec_time_ns_or_None)."""
    x = np.asarray(inputs["x"], dtype=np.float32)
    w_qkv = np.asarray(inputs["w_qkv"], dtype=np.float32)
    b_qkv = np.asarray(inputs["b_qkv"], dtype=np.float32)
    w_out = np.asarray(inputs["w_out"], dtype=np.float32)
    b_out = np.asarray(inputs["b_out"], dtype=np.float32)

    nc = _get_nc()
    in_maps = _make_in_maps(x, w_qkv, b_qkv, w_out)
    res = run_bass_kernel_spmd(nc, in_maps, list(range(NCORES)), trace=trace)
    part = np.zeros((NTT, 128, C), dtype=np.float32)
    for r in res.results:
        part += r["part"]
    # bv is not applied on-device: softmax weights sum to 1, so it adds
    # bv @ w_out to every row — fold it in here with the output bias
    y = part.reshape(BT, C) + (b_out + b_qkv[2 * C :] @ w_out)[None, :]
    return y.reshape(B, T, C).astype(np.float32), res.exec_time_ns


def kernel(**inputs):
    return run(inputs, trace=False)[0]

